# revision 20
# baseline (speedup 1.0000x reference)
"""Trainium2 Bass kernel for BSplineBasis (cubic, 64 clamped knots, 60 basis fns).

v3 design (vs baseline):
  - No collective: every core reduces the FULL x locally (identical inputs +
    identical instruction order -> bit-identical normalization on all cores).
    The heavy lifting is done by accumulating DMAs (accum_op=max/min over the
    8 row-shards), so VectorE only reduces a [128,1024] tile.
  - z-broadcast via ONE fp16 matmul per PSUM chunk: lhsT = [vhiT; vloT; ones]
    (65 rows), rhs = [delta; delta; crow].  The hi/lo split keeps v's
    precision at ~2^-22 while running the PE at 16-bit rate (the baseline ran
    2 fp32 matmuls = 8 PE passes per chunk).  vhi/vlo transposed by xbar DMA.
  - Dense elementwise in fp16: ScalarE does Abs + the two Squares, VectorE
    does the clamps (4x TS) and three 2x TTs.  No STT on the hot path (STT
    has no 2x uop), no GpSimd (Q7 f32->f16 runs ~18cyc/elem and starves the
    shared SBUF port).
  - Output DRAM tensor is fp16 (halves store traffic); host upcasts to f32.

Math per dense element (a = |z|, z = v - c + 1):
  B(z) = (2-a)+^3/6 - 4(1-a)+^3/6 = T2 - T1'
  An = min(a-2,0), Bn = min(a-1,0)
  A2 = (QS*(2-a))^2, B2 = (2QS*(1-a))^2, QS = 6^-1/2 (unclamped squares; the
  clamped partner factor zeroes them outside their support)
  T1' = An*A2 = -(2-a)+^3/6,  T2 = Bn*B2 = -4(1-a)+^3/6
Rows with x == gmax get v += 100 so every basis value evaluates to 0
(matches the reference's half-open binning); fp16-safe since a <= ~160.
The 6 boundary columns (repeated end knots) are exact combinations of
truncated cubes r_k = (k-v)+ (left; right side identical in w = 57 - v):
  col0 = r1^3, col1 = -2 r1^3 + r2^3/4, col2 = 1.5 r1^3 - 0.75 r2^3 + r3^3/6
computed as c1 = r1*r1^2, c2 = r2*(r2/2)^2, c3 = r3*(QS*r3)^2 and two
accumulating STTs, overwritten into the output tiles.
"""
import sys

for _p in ("/opt/trn_rl_repo",):
    if _p not in sys.path:
        sys.path.insert(0, _p)

from contextlib import ExitStack

import numpy as np

import concourse.bass as bass
import concourse.mybir as mybir
import concourse.tile as tile
from concourse.bass_utils import run_bass_kernel_spmd

N_CORES = 8
N_ROWS = 1048576
C = 60
P = 128
RPC = N_ROWS // N_CORES                    # 131072 rows per core
IPP = RPC // P                             # 1024 rows per partition
R = 32                                     # rows per dense tile (per partition)
NT = IPP // R                              # 32 dense tiles per core
F = R * C                                  # 1920 dense free elems per tile
KK = 2 * R + 1                             # 65 contraction rows (hi, lo, ones)
QS = float(1.0 / np.sqrt(6.0))
BIG = 100.0                                # fp16-safe mask offset

AL = mybir.AluOpType
FT = mybir.ActivationFunctionType

_CACHE = {}


def _consts():
    ones1 = np.ones((1, P), dtype=np.float32)
    # delta[r, (r2, c)] = 1 if r == r2 else 0   -> replicates v along c
    delta = np.kron(np.eye(R, dtype=np.float32), np.ones((1, C), np.float32))
    # crow[(r, c)] = 1 - c                      -> z = v - c + 1
    crow = np.tile((1.0 - np.arange(C, dtype=np.float32))[None, :], (1, R))
    dcat = np.concatenate([delta, delta, crow], axis=0).astype(np.float16)
    return ones1, dcat


def _split_multiwait_ctrl(nc, max_waits=1):
    """walrus rejects CTRL instructions carrying >~2 sem waits; move excess
    waits onto same-engine NoOps inserted right before (same semantics)."""
    for f in nc.m.functions:
        for blk in f.blocks:
            il = blk.instructions
            out = []
            changed = False
            for inst in il:
                si = inst.sync_info
                if (
                    si is not None
                    and si.on_wait
                    and len(si.on_wait) > max_waits
                    and type(inst).__name__
                    not in ("InstEventSemaphore", "InstUnconditionalBranch")
                ):
                    waits = list(si.on_wait)
                    extra, keep = waits[:-max_waits], waits[-max_waits:]
                    for w in extra:
                        nop = mybir.InstNoOp(
                            name=f"I-splitw-{nc.next_id()}", ins=[], outs=[]
                        )
                        nop.engine = inst.engine
                        nop.sync_info = mybir.SyncInfo(on_wait=[w], on_update=[])
                        out.append(nop)
                        nc.register_instruction(nop)
                    inst.sync_info = mybir.SyncInfo(
                        on_wait=keep, on_update=list(si.on_update or [])
                    )
                    changed = True
                out.append(inst)
            if changed:
                il[:] = out


def build():
    f32 = mybir.dt.float32
    f16 = mybir.dt.float16
    nc = bass.Bass(trn_type="TRN2", num_devices=N_CORES)
    x_h = nc.declare_dram_parameter("x", [RPC, 1], f32, isOutput=False)
    xf_h = nc.declare_dram_parameter("xfull", [N_ROWS, 1], f32, isOutput=False)
    ones_h = nc.declare_dram_parameter("ones1", [1, P], f32, isOutput=False)
    dcat_h = nc.declare_dram_parameter("dcat", [KK, F], f16, isOutput=False)
    y_h = nc.declare_dram_parameter("y", [RPC, C], f16, isOutput=True)

    x_ap = x_h[:].rearrange("(p i) o -> p (i o)", p=P)            # [128, 1024]
    xf_ap = xf_h[:].rearrange("(p i) o -> p (i o)", p=P)          # [128, 8192]
    y_ap = y_h[:].rearrange("(p i) c -> p (i c)", p=P)            # [128, 1024*60]

    with tile.TileContext(nc) as tc, ExitStack() as ctx:
        cpool = ctx.enter_context(tc.tile_pool(name="consts", bufs=1))
        spool = ctx.enter_context(tc.tile_pool(name="small", bufs=1))
        dpool = ctx.enter_context(tc.tile_pool(name="dense", bufs=2))
        opool = ctx.enter_context(tc.tile_pool(name="outp", bufs=2))
        bpool = ctx.enter_context(tc.tile_pool(name="bnd", bufs=1))
        pp = ctx.enter_context(tc.tile_pool(name="ps", bufs=1, space="PSUM"))

        # ---- global min/max: every core reduces the FULL x ----
        # (identical inputs + identical op order -> bit-identical
        # normalization on all cores, no collective needed).  The 4MB load is
        # chunked so the reduces pipeline with the DMA; it goes first so the
        # critical path starts immediately.
        NCH = 4
        CW = (N_ROWS // P) // NCH  # 2048
        xf = cpool.tile([P, NCH * CW], f32)
        for s in range(NCH):
            sl = slice(CW * s, CW * (s + 1))
            nc.sync.dma_start(xf[:, sl], xf_ap[:, sl])
        xt = cpool.tile([P, IPP], f32)
        nc.sync.dma_start(xt[:], x_ap)
        ones1 = cpool.tile([1, P], f32)
        nc.sync.dma_start(ones1[:], ones_h[:])
        dcat = cpool.tile([KK, F], f16)
        nc.sync.dma_start(dcat[:], dcat_h[:])
        cmb = cpool.tile([KK, NT * P], f16)
        nc.vector.memset(cmb[2 * R : KK, :], 1.0)
        bconst = spool.tile([P, 1], f32)
        nc.vector.memset(bconst[:], 2.0 * QS)  # bias for both dense Squares
        kbias = spool.tile([P, 3], f32)        # biases for boundary Relus
        for i in range(3):
            nc.vector.memset(kbias[:, i : i + 1], float(i + 1))

        mxp = spool.tile([P, NCH], f32)
        mnp = spool.tile([P, NCH], f32)
        for s in range(NCH):
            sl = slice(CW * s, CW * (s + 1))
            nc.vector.tensor_reduce(
                mxp[:, s : s + 1], xf[:, sl], axis=mybir.AxisListType.X, op=AL.max
            )
            nc.vector.tensor_reduce(
                mnp[:, s : s + 1], xf[:, sl], axis=mybir.AxisListType.X, op=AL.min
            )
        mx = spool.tile([P, 1], f32)
        nc.vector.tensor_reduce(mx[:], mxp[:], axis=mybir.AxisListType.X, op=AL.max)
        mn = spool.tile([P, 1], f32)
        nc.vector.tensor_reduce(mn[:], mnp[:], axis=mybir.AxisListType.X, op=AL.min)
        pk = spool.tile([P, 2], f32)
        nc.vector.tensor_scalar(pk[:, 0:1], mn[:], -1.0, None, AL.mult)
        nc.vector.tensor_copy(pk[:, 1:2], mx[:])
        g2 = spool.tile([1, 2], f32)
        nc.gpsimd.tensor_reduce(g2[:], pk[:], axis=mybir.AxisListType.C, op=AL.max)
        # g2 = [-gmin, gmax]

        # ---- scalar params: k, b, c2, gmax, -k ----
        sc = spool.tile([1, 8], f32)
        neg_gmin = g2[:, 0:1]
        gmax1 = g2[:, 1:2]
        rng_ = spool.tile([1, 1], f32)
        nc.vector.tensor_tensor(rng_[:], gmax1, neg_gmin, AL.add)
        den = spool.tile([1, 1], f32)
        nc.vector.tensor_scalar(den[:], rng_[:], 1.0e-8, None, AL.add)
        rcp = spool.tile([1, 1], f32)
        nc.vector.reciprocal(rcp[:], den[:])
        nc.vector.tensor_scalar(sc[:, 0:1], rcp[:], 57.0, None, AL.mult)  # k
        nc.vector.tensor_tensor(sc[:, 1:2], neg_gmin, sc[:, 0:1], AL.mult)  # b = -gmin*k
        nc.vector.tensor_scalar(sc[:, 2:3], sc[:, 1:2], -1.0, 57.0, AL.mult, AL.add)  # c2 = 57-b
        nc.vector.tensor_copy(sc[:, 3:4], gmax1)  # gmax
        nc.vector.tensor_scalar(sc[:, 4:5], sc[:, 0:1], -1.0, None, AL.mult)  # -k

        # broadcast scalars to all partitions via PE (reuses the zps PSUM
        # buffer; the first dense matmul serializes behind the copy-out)
        bc_ps = pp.tile([P, 8], f32, tag="zps")
        nc.tensor.matmul(bc_ps[:], ones1[:], sc[:], start=True, stop=True)
        bc = spool.tile([P, 8], f32)
        nc.scalar.copy(bc[:], bc_ps[:])
        K_, B_, C2_, GM_, NK_ = (bc[:, i : i + 1] for i in range(5))

        # ---- v (fp32, row-major layout) ----
        vr = cpool.tile([P, IPP], f32)
        msk = cpool.tile([P, IPP], f32)
        v = cpool.tile([P, IPP], f32)
        nc.vector.tensor_scalar(vr[:], xt[:], K_, B_, AL.mult, AL.add)      # x*k+b
        nc.vector.tensor_scalar(msk[:], xt[:], GM_, None, AL.is_equal)
        nc.vector.scalar_tensor_tensor(v[:], msk[:], BIG, vr[:], AL.mult, AL.add)

        # hi/lo fp16 split of v (exact to ~2^-22)
        vhi = cpool.tile([P, IPP], f16)
        nc.vector.tensor_copy(vhi[:], v[:])
        vlo = cpool.tile([P, IPP], f16)
        nc.vector.tensor_tensor(vlo[:], v[:], vhi[:], AL.subtract)

        # ---- transpose vhi/vlo (xbar DMA) and assemble combined lhsT ----
        # vT[a, (j p)] = v[p, 128j + a]  (8 transposed 128x128 blocks);
        # cmb[r, 128t + p] = vT[32*(t%4) + r, 128*(t//4) + p], one strided DMA
        # per (half, j) so dense tile 4j can start as soon as block j is done.
        vhiT = cpool.tile([P, IPP], f16)
        vloT = cpool.tile([P, IPP], f16)
        dst_hi = cmb[0:R, :].rearrange("r (j k p) -> r j k p", k=4, p=P)
        dst_lo = cmb[R : 2 * R, :].rearrange("r (j k p) -> r j k p", k=4, p=P)
        NB = IPP // P  # 8
        for j in range(NB):
            s = slice(P * j, P * (j + 1))
            nc.sync.dma_start_transpose(vhiT[:, s], vhi[:, s])
            nc.sync.dma_start_transpose(vloT[:, s], vlo[:, s])
            src_hi = vhiT[:, s].rearrange("(k r) p -> r k p", r=R)
            src_lo = vloT[:, s].rearrange("(k r) p -> r k p", r=R)
            nc.sync.dma_start(dst_hi[:, j, :, :], src_hi)
            nc.sync.dma_start(dst_lo[:, j, :, :], src_lo)

        # ---- w (for the right-edge boundary columns) ----
        wr = cpool.tile([P, IPP], f32)
        w = cpool.tile([P, IPP], f32)
        nc.vector.tensor_scalar(wr[:], xt[:], NK_, C2_, AL.mult, AL.add)    # 57-v
        nc.vector.scalar_tensor_tensor(w[:], msk[:], BIG, wr[:], AL.mult, AL.add)

        # ---- boundary columns (fp16, column-major [P, (slot, i)]) ----
        # L slot s holds output col s; R slot s holds output col 57+s.
        L = bpool.tile([P, 3 * IPP], f16)
        Rt = bpool.tile([P, 3 * IPP], f16)

        def phase_boundary():
            for side, src, dst, jmap in (
                ("L", v, L, lambda j: j),
                ("R", w, Rt, lambda j: 2 - j),
            ):
                def dslot(j):
                    sl = jmap(j)
                    return dst[:, sl * IPP : (sl + 1) * IPP]

                rs = []
                qs = []
                for ki, sqs in ((0, 1.0), (1, 0.5), (2, QS)):
                    r_ = bpool.tile([P, IPP], f16, tag=f"r{ki}", bufs=2)
                    nc.scalar.activation(
                        r_[:], src[:], FT.Relu, bias=kbias[:, ki : ki + 1], scale=-1.0
                    )
                    rs.append(r_)
                    q_ = bpool.tile([P, IPP], f16, tag=f"q{ki}", bufs=2)
                    nc.scalar.activation(q_[:], r_[:], FT.Square, scale=sqs)
                    qs.append(q_)
                # c1 = r1^3 -> col0 directly
                nc.vector.tensor_tensor(dslot(0), rs[0][:], qs[0][:], AL.mult)
                c2 = bpool.tile([P, IPP], f16, tag="c2", bufs=2)
                nc.vector.tensor_tensor(c2[:], rs[1][:], qs[1][:], AL.mult)  # r2^3/4
                c3 = bpool.tile([P, IPP], f16, tag="c3", bufs=2)
                nc.vector.tensor_tensor(c3[:], rs[2][:], qs[2][:], AL.mult)  # r3^3/6
                # col1 = -2*c1 + c2
                nc.vector.scalar_tensor_tensor(
                    dslot(1), dslot(0), -2.0, c2[:], AL.mult, AL.add
                )
                # col2 = 1.5*c1 - 3*c2 + c3
                d_ = bpool.tile([P, IPP], f16, tag="d", bufs=2)
                nc.vector.scalar_tensor_tensor(d_[:], c2[:], -3.0, c3[:], AL.mult, AL.add)
                nc.vector.scalar_tensor_tensor(
                    dslot(2), dslot(0), 1.5, d_[:], AL.mult, AL.add
                )

        phase_boundary()
        Lv = L[:].rearrange("p (c i) -> p i c", c=3)  # [P, IPP, 3]
        Rv = Rt[:].rearrange("p (c i) -> p i c", c=3)

        # ---- dense tiles ----
        bnds = list(range(0, F, 512)) + [F]  # PSUM-bank-aligned chunks

        def dense_tile(t):
            lhsT = cmb[:, P * t : P * (t + 1)]  # [65, 128]
            z = pp.tile([P, F], f32, tag="zps")
            for q in range(len(bnds) - 1):
                s = slice(bnds[q], bnds[q + 1])
                nc.tensor.matmul(z[:, s], lhsT, dcat[:, s], start=True, stop=True)

            a = dpool.tile([P, F], f16, tag="a", bufs=3)
            nc.scalar.activation(a[:], z[:], FT.Abs)
            A2 = dpool.tile([P, F], f16, tag="A2")
            nc.scalar.activation(A2[:], a[:], FT.Square, bias=bconst[:], scale=-QS)
            B2 = dpool.tile([P, F], f16, tag="B2")
            nc.scalar.activation(B2[:], a[:], FT.Square, bias=bconst[:], scale=-2.0 * QS)
            An = dpool.tile([P, F], f16, tag="An")
            nc.vector.tensor_scalar(An[:], a[:], 2.0, 0.0, AL.subtract, AL.min)
            Bn = dpool.tile([P, F], f16, tag="Bn")
            nc.vector.tensor_scalar(Bn[:], a[:], 1.0, 0.0, AL.subtract, AL.min)
            T1 = dpool.tile([P, F], f16, tag="T1")
            nc.vector.tensor_tensor(T1[:], An[:], A2[:], AL.mult)   # = -p^3/6
            T2 = dpool.tile([P, F], f16, tag="T2")
            nc.vector.tensor_tensor(T2[:], Bn[:], B2[:], AL.mult)   # = -4q^3/6
            ot = opool.tile([P, F], f16, tag="ot")
            nc.vector.tensor_tensor(ot[:], T2[:], T1[:], AL.subtract)

            o3 = ot[:].rearrange("p (r c) -> p r c", c=C)
            nc.vector.tensor_copy(o3[:, :, 0:3], Lv[:, t * R : (t + 1) * R, :])
            nc.vector.tensor_copy(o3[:, :, 57:60], Rv[:, t * R : (t + 1) * R, :])
            nc.sync.dma_start(y_ap[:, t * F : (t + 1) * F], ot[:])

        for t in range(NT):
            dense_tile(t)

    _split_multiwait_ctrl(nc)
    return nc


def kernel(x, knots=None):
    x = np.ascontiguousarray(np.asarray(x, dtype=np.float32).reshape(N_ROWS, 1))
    if "nc" not in _CACHE:
        _CACHE["nc"] = build()
    nc = _CACHE["nc"]
    ones1, dcat = _consts()
    in_maps = []
    for c in range(N_CORES):
        in_maps.append(
            {
                "x": x[c * RPC : (c + 1) * RPC],
                "xfull": x,
                "ones1": ones1,
                "dcat": dcat,
            }
        )
    res = run_bass_kernel_spmd(nc, in_maps, list(range(N_CORES)))
    _CACHE["last_result"] = res
    return np.concatenate(
        [res.results[c]["y"].astype(np.float32) for c in range(N_CORES)], axis=0
    )


# revision 21
# speedup vs baseline: 1.0345x; 1.0345x over previous
"""Trainium2 Bass kernel for BSplineBasis (cubic, 64 clamped knots, 60 basis fns).

v3 design (vs baseline):
  - No collective: every core reduces the FULL x locally (identical inputs +
    identical instruction order -> bit-identical normalization on all cores).
    The heavy lifting is done by accumulating DMAs (accum_op=max/min over the
    8 row-shards), so VectorE only reduces a [128,1024] tile.
  - z-broadcast via ONE fp16 matmul per PSUM chunk: lhsT = [vhiT; vloT; ones]
    (65 rows), rhs = [delta; delta; crow].  The hi/lo split keeps v's
    precision at ~2^-22 while running the PE at 16-bit rate (the baseline ran
    2 fp32 matmuls = 8 PE passes per chunk).  vhi/vlo transposed by xbar DMA.
  - Dense elementwise in fp16: ScalarE does Abs + the two Squares, VectorE
    does the clamps (4x TS) and three 2x TTs.  No STT on the hot path (STT
    has no 2x uop), no GpSimd (Q7 f32->f16 runs ~18cyc/elem and starves the
    shared SBUF port).
  - Output DRAM tensor is fp16 (halves store traffic); host upcasts to f32.

Math per dense element (a = |z|, z = v - c + 1):
  B(z) = (2-a)+^3/6 - 4(1-a)+^3/6 = T2 - T1'
  An = min(a-2,0), Bn = min(a-1,0)
  A2 = (QS*(2-a))^2, B2 = (2QS*(1-a))^2, QS = 6^-1/2 (unclamped squares; the
  clamped partner factor zeroes them outside their support)
  T1' = An*A2 = -(2-a)+^3/6,  T2 = Bn*B2 = -4(1-a)+^3/6
Rows with x == gmax get v += 100 so every basis value evaluates to 0
(matches the reference's half-open binning); fp16-safe since a <= ~160.
The 6 boundary columns (repeated end knots) are exact combinations of
truncated cubes r_k = (k-v)+ (left; right side identical in w = 57 - v):
  col0 = r1^3, col1 = -2 r1^3 + r2^3/4, col2 = 1.5 r1^3 - 0.75 r2^3 + r3^3/6
computed as c1 = r1*r1^2, c2 = r2*(r2/2)^2, c3 = r3*(QS*r3)^2 and two
accumulating STTs, overwritten into the output tiles.
"""
import sys

for _p in ("/opt/trn_rl_repo",):
    if _p not in sys.path:
        sys.path.insert(0, _p)

from contextlib import ExitStack

import numpy as np

import concourse.bass as bass
import concourse.mybir as mybir
import concourse.tile as tile
from concourse.bass_utils import run_bass_kernel_spmd

N_CORES = 8
N_ROWS = 1048576
C = 60
P = 128
RPC = N_ROWS // N_CORES                    # 131072 rows per core
IPP = RPC // P                             # 1024 rows per partition
R = 32                                     # rows per dense tile (per partition)
NT = IPP // R                              # 32 dense tiles per core
F = R * C                                  # 1920 dense free elems per tile
KK = 2 * R + 1                             # 65 contraction rows (hi, lo, ones)
QS = float(1.0 / np.sqrt(6.0))
BIG = 100.0                                # fp16-safe mask offset

AL = mybir.AluOpType
FT = mybir.ActivationFunctionType

_CACHE = {}


def _consts():
    ones1 = np.ones((1, P), dtype=np.float32)
    # delta[r, (r2, c)] = 1 if r == r2 else 0   -> replicates v along c
    delta = np.kron(np.eye(R, dtype=np.float32), np.ones((1, C), np.float32))
    # crow[(r, c)] = 1 - c                      -> z = v - c + 1
    crow = np.tile((1.0 - np.arange(C, dtype=np.float32))[None, :], (1, R))
    dcat = np.concatenate([delta, delta, crow], axis=0).astype(np.float16)
    return ones1, dcat


def _split_multiwait_ctrl(nc, max_waits=1):
    """walrus rejects CTRL instructions carrying >~2 sem waits; move excess
    waits onto same-engine NoOps inserted right before (same semantics)."""
    for f in nc.m.functions:
        for blk in f.blocks:
            il = blk.instructions
            out = []
            changed = False
            for inst in il:
                si = inst.sync_info
                if (
                    si is not None
                    and si.on_wait
                    and len(si.on_wait) > max_waits
                    and type(inst).__name__
                    not in ("InstEventSemaphore", "InstUnconditionalBranch")
                ):
                    waits = list(si.on_wait)
                    extra, keep = waits[:-max_waits], waits[-max_waits:]
                    for w in extra:
                        nop = mybir.InstNoOp(
                            name=f"I-splitw-{nc.next_id()}", ins=[], outs=[]
                        )
                        nop.engine = inst.engine
                        nop.sync_info = mybir.SyncInfo(on_wait=[w], on_update=[])
                        out.append(nop)
                        nc.register_instruction(nop)
                    inst.sync_info = mybir.SyncInfo(
                        on_wait=keep, on_update=list(si.on_update or [])
                    )
                    changed = True
                out.append(inst)
            if changed:
                il[:] = out


def build():
    f32 = mybir.dt.float32
    f16 = mybir.dt.float16
    nc = bass.Bass(trn_type="TRN2", num_devices=N_CORES)
    x_h = nc.declare_dram_parameter("x", [RPC, 1], f32, isOutput=False)
    xf_h = nc.declare_dram_parameter("xfull", [N_ROWS, 1], f32, isOutput=False)
    ones_h = nc.declare_dram_parameter("ones1", [1, P], f32, isOutput=False)
    dcat_h = nc.declare_dram_parameter("dcat", [KK, F], f16, isOutput=False)
    y_h = nc.declare_dram_parameter("y", [RPC, C], f16, isOutput=True)

    x_ap = x_h[:].rearrange("(p i) o -> p (i o)", p=P)            # [128, 1024]
    xf_ap = xf_h[:].rearrange("(p i) o -> p (i o)", p=P)          # [128, 8192]
    y_ap = y_h[:].rearrange("(p i) c -> p (i c)", p=P)            # [128, 1024*60]

    with tile.TileContext(nc) as tc, ExitStack() as ctx:
        cpool = ctx.enter_context(tc.tile_pool(name="consts", bufs=1))
        spool = ctx.enter_context(tc.tile_pool(name="small", bufs=1))
        dpool = ctx.enter_context(tc.tile_pool(name="dense", bufs=2))
        opool = ctx.enter_context(tc.tile_pool(name="outp", bufs=2))
        bpool = ctx.enter_context(tc.tile_pool(name="bnd", bufs=1))
        pp = ctx.enter_context(tc.tile_pool(name="ps", bufs=1, space="PSUM"))

        # ---- global min/max: every core reduces the FULL x ----
        # (identical inputs + identical op order -> bit-identical
        # normalization on all cores, no collective needed).  The 4MB load is
        # chunked so the reduces pipeline with the DMA; it goes first so the
        # critical path starts immediately.
        NCH = 4
        CW = (N_ROWS // P) // NCH  # 2048
        xf = cpool.tile([P, NCH * CW], f32)
        for s in range(NCH):
            sl = slice(CW * s, CW * (s + 1))
            nc.sync.dma_start(xf[:, sl], xf_ap[:, sl])
        xt = cpool.tile([P, IPP], f32)
        nc.sync.dma_start(xt[:], x_ap)
        ones1 = cpool.tile([1, P], f32)
        nc.sync.dma_start(ones1[:], ones_h[:])
        dcat = cpool.tile([KK, F], f16)
        nc.sync.dma_start(dcat[:], dcat_h[:])
        cmb = cpool.tile([KK, NT * P], f16)
        nc.vector.memset(cmb[2 * R : KK, :], 1.0)
        bconst = spool.tile([P, 1], f32)
        nc.vector.memset(bconst[:], 2.0 * QS)  # bias for both dense Squares
        kbias = spool.tile([P, 3], f32)        # biases for boundary Relus
        for i in range(3):
            nc.vector.memset(kbias[:, i : i + 1], float(i + 1))

        mxp = spool.tile([P, NCH], f32)
        mnp = spool.tile([P, NCH], f32)
        for s in range(NCH):
            sl = slice(CW * s, CW * (s + 1))
            nc.vector.tensor_reduce(
                mxp[:, s : s + 1], xf[:, sl], axis=mybir.AxisListType.X, op=AL.max
            )
            nc.vector.tensor_reduce(
                mnp[:, s : s + 1], xf[:, sl], axis=mybir.AxisListType.X, op=AL.min
            )
        mx = spool.tile([P, 1], f32)
        nc.vector.tensor_reduce(mx[:], mxp[:], axis=mybir.AxisListType.X, op=AL.max)
        mn = spool.tile([P, 1], f32)
        nc.vector.tensor_reduce(mn[:], mnp[:], axis=mybir.AxisListType.X, op=AL.min)
        pk = spool.tile([P, 2], f32)
        nc.vector.tensor_scalar(pk[:, 0:1], mn[:], -1.0, None, AL.mult)
        nc.vector.tensor_copy(pk[:, 1:2], mx[:])
        g2 = spool.tile([1, 2], f32)
        nc.gpsimd.tensor_reduce(g2[:], pk[:], axis=mybir.AxisListType.C, op=AL.max)
        # g2 = [-gmin, gmax]

        # ---- scalar params: k, b, c2, gmax, -k ----
        sc = spool.tile([1, 8], f32)
        neg_gmin = g2[:, 0:1]
        gmax1 = g2[:, 1:2]
        rng_ = spool.tile([1, 1], f32)
        nc.vector.tensor_tensor(rng_[:], gmax1, neg_gmin, AL.add)
        den = spool.tile([1, 1], f32)
        nc.vector.tensor_scalar(den[:], rng_[:], 1.0e-8, None, AL.add)
        rcp = spool.tile([1, 1], f32)
        nc.vector.reciprocal(rcp[:], den[:])
        nc.vector.tensor_scalar(sc[:, 0:1], rcp[:], 57.0, None, AL.mult)  # k
        nc.vector.tensor_tensor(sc[:, 1:2], neg_gmin, sc[:, 0:1], AL.mult)  # b = -gmin*k
        nc.vector.tensor_scalar(sc[:, 2:3], sc[:, 1:2], -1.0, 57.0, AL.mult, AL.add)  # c2 = 57-b
        nc.vector.tensor_copy(sc[:, 3:4], gmax1)  # gmax
        nc.vector.tensor_scalar(sc[:, 4:5], sc[:, 0:1], -1.0, None, AL.mult)  # -k

        # broadcast scalars to all partitions via PE (reuses the zps PSUM
        # buffer; the first dense matmul serializes behind the copy-out)
        bc_ps = pp.tile([P, 8], f32, tag="zps")
        nc.tensor.matmul(bc_ps[:], ones1[:], sc[:], start=True, stop=True)
        bc = spool.tile([P, 8], f32)
        nc.scalar.copy(bc[:], bc_ps[:])
        K_, B_, C2_, GM_, NK_ = (bc[:, i : i + 1] for i in range(5))

        # ---- v (fp32, row-major layout) ----
        vr = cpool.tile([P, IPP], f32)
        msk = cpool.tile([P, IPP], f32)
        v = cpool.tile([P, IPP], f32)
        nc.vector.tensor_scalar(vr[:], xt[:], K_, B_, AL.mult, AL.add)      # x*k+b
        nc.vector.tensor_scalar(msk[:], xt[:], GM_, None, AL.is_equal)
        nc.vector.scalar_tensor_tensor(v[:], msk[:], BIG, vr[:], AL.mult, AL.add)

        # hi/lo fp16 split of v (exact to ~2^-22)
        vhi = cpool.tile([P, IPP], f16)
        nc.vector.tensor_copy(vhi[:], v[:])
        vlo = cpool.tile([P, IPP], f16)
        nc.vector.tensor_tensor(vlo[:], v[:], vhi[:], AL.subtract)

        # ---- transpose vhi/vlo (xbar DMA) and assemble combined lhsT ----
        # vT[a, (j p)] = v[p, 128j + a]  (8 transposed 128x128 blocks);
        # cmb[r, 128t + p] = vT[32*(t%4) + r, 128*(t//4) + p], one strided DMA
        # per (half, j) so dense tile 4j can start as soon as block j is done.
        vhiT = cpool.tile([P, IPP], f16)
        vloT = cpool.tile([P, IPP], f16)
        dst_hi = cmb[0:R, :].rearrange("r (j k p) -> r j k p", k=4, p=P)
        dst_lo = cmb[R : 2 * R, :].rearrange("r (j k p) -> r j k p", k=4, p=P)
        NB = IPP // P  # 8
        for j in range(NB):
            s = slice(P * j, P * (j + 1))
            nc.sync.dma_start_transpose(vhiT[:, s], vhi[:, s])
            nc.sync.dma_start_transpose(vloT[:, s], vlo[:, s])
            src_hi = vhiT[:, s].rearrange("(k r) p -> r k p", r=R)
            src_lo = vloT[:, s].rearrange("(k r) p -> r k p", r=R)
            nc.sync.dma_start(dst_hi[:, j, :, :], src_hi)
            nc.sync.dma_start(dst_lo[:, j, :, :], src_lo)

        # ---- w (for the right-edge boundary columns) ----
        wr = cpool.tile([P, IPP], f32)
        w = cpool.tile([P, IPP], f32)
        nc.vector.tensor_scalar(wr[:], xt[:], NK_, C2_, AL.mult, AL.add)    # 57-v
        nc.vector.scalar_tensor_tensor(w[:], msk[:], BIG, wr[:], AL.mult, AL.add)

        # ---- boundary columns (fp16, column-major [P, (slot, i)]) ----
        # L slot s holds output col s; R slot s holds output col 57+s.
        L = bpool.tile([P, 3 * IPP], f16)
        Rt = bpool.tile([P, 3 * IPP], f16)

        def phase_boundary():
            for side, src, dst, jmap in (
                ("L", v, L, lambda j: j),
                ("R", w, Rt, lambda j: 2 - j),
            ):
                def dslot(j):
                    sl = jmap(j)
                    return dst[:, sl * IPP : (sl + 1) * IPP]

                rs = []
                qs = []
                for ki, sqs in ((0, 1.0), (1, 0.5), (2, QS)):
                    r_ = bpool.tile([P, IPP], f16, tag=f"r{ki}", bufs=2)
                    nc.scalar.activation(
                        r_[:], src[:], FT.Relu, bias=kbias[:, ki : ki + 1], scale=-1.0
                    )
                    rs.append(r_)
                    q_ = bpool.tile([P, IPP], f16, tag=f"q{ki}", bufs=2)
                    nc.scalar.activation(q_[:], r_[:], FT.Square, scale=sqs)
                    qs.append(q_)
                # c1 = r1^3 -> col0 directly
                nc.vector.tensor_tensor(dslot(0), rs[0][:], qs[0][:], AL.mult)
                c2 = bpool.tile([P, IPP], f16, tag="c2", bufs=2)
                nc.vector.tensor_tensor(c2[:], rs[1][:], qs[1][:], AL.mult)  # r2^3/4
                c3 = bpool.tile([P, IPP], f16, tag="c3", bufs=2)
                nc.vector.tensor_tensor(c3[:], rs[2][:], qs[2][:], AL.mult)  # r3^3/6
                # col1 = -2*c1 + c2
                nc.vector.scalar_tensor_tensor(
                    dslot(1), dslot(0), -2.0, c2[:], AL.mult, AL.add
                )
                # col2 = 1.5*c1 - 3*c2 + c3
                d_ = bpool.tile([P, IPP], f16, tag="d", bufs=2)
                nc.vector.scalar_tensor_tensor(d_[:], c2[:], -3.0, c3[:], AL.mult, AL.add)
                nc.vector.scalar_tensor_tensor(
                    dslot(2), dslot(0), 1.5, d_[:], AL.mult, AL.add
                )

        phase_boundary()
        Lv = L[:].rearrange("p (c i) -> p i c", c=3)  # [P, IPP, 3]
        Rv = Rt[:].rearrange("p (c i) -> p i c", c=3)

        # ---- dense tiles, two per elementwise pass ----
        bnds = list(range(0, F, 512)) + [F]  # PSUM-bank-aligned chunks
        F2 = 2 * F

        def dense_pair(pr):
            zz = []
            for h in range(2):
                t = 2 * pr + h
                lhsT = cmb[:, P * t : P * (t + 1)]  # [65, 128]
                z = pp.tile([P, F], f32, tag=f"zps{h}" if h else "zps")
                for q in range(len(bnds) - 1):
                    s = slice(bnds[q], bnds[q + 1])
                    nc.tensor.matmul(z[:, s], lhsT, dcat[:, s], start=True, stop=True)
                zz.append(z)

            a = dpool.tile([P, F2], f16, tag="a")
            nc.scalar.activation(a[:, 0:F], zz[0][:], FT.Abs)
            nc.scalar.activation(a[:, F:F2], zz[1][:], FT.Abs)
            A2 = dpool.tile([P, F2], f16, tag="A2", bufs=1)
            nc.scalar.activation(A2[:], a[:], FT.Square, bias=bconst[:], scale=-QS)
            B2 = dpool.tile([P, F2], f16, tag="B2", bufs=1)
            nc.scalar.activation(B2[:], a[:], FT.Square, bias=bconst[:], scale=-2.0 * QS)
            An = dpool.tile([P, F2], f16, tag="An", bufs=1)
            nc.vector.tensor_scalar(An[:], a[:], 2.0, 0.0, AL.subtract, AL.min)
            Bn = dpool.tile([P, F2], f16, tag="Bn", bufs=1)
            nc.vector.tensor_scalar(Bn[:], a[:], 1.0, 0.0, AL.subtract, AL.min)
            T1 = dpool.tile([P, F2], f16, tag="T1", bufs=1)
            nc.vector.tensor_tensor(T1[:], An[:], A2[:], AL.mult)   # = -p^3/6
            T2 = dpool.tile([P, F2], f16, tag="T2", bufs=1)
            nc.vector.tensor_tensor(T2[:], Bn[:], B2[:], AL.mult)   # = -4q^3/6
            ot = opool.tile([P, F2], f16, tag="ot")
            nc.vector.tensor_tensor(ot[:], T2[:], T1[:], AL.subtract)

            o3 = ot[:].rearrange("p (r c) -> p r c", c=C)
            nc.vector.tensor_copy(o3[:, :, 0:3], Lv[:, pr * 2 * R : (pr + 1) * 2 * R, :])
            nc.vector.tensor_copy(o3[:, :, 57:60], Rv[:, pr * 2 * R : (pr + 1) * 2 * R, :])
            nc.sync.dma_start(y_ap[:, pr * F2 : (pr + 1) * F2], ot[:])

        for pr in range(NT // 2):
            dense_pair(pr)

    _split_multiwait_ctrl(nc)
    return nc


def kernel(x, knots=None):
    x = np.ascontiguousarray(np.asarray(x, dtype=np.float32).reshape(N_ROWS, 1))
    if "nc" not in _CACHE:
        _CACHE["nc"] = build()
    nc = _CACHE["nc"]
    ones1, dcat = _consts()
    in_maps = []
    for c in range(N_CORES):
        in_maps.append(
            {
                "x": x[c * RPC : (c + 1) * RPC],
                "xfull": x,
                "ones1": ones1,
                "dcat": dcat,
            }
        )
    res = run_bass_kernel_spmd(nc, in_maps, list(range(N_CORES)))
    _CACHE["last_result"] = res
    return np.concatenate(
        [res.results[c]["y"].astype(np.float32) for c in range(N_CORES)], axis=0
    )
